# revision 5
# baseline (speedup 1.0000x reference)
"""DistanceConv2 GNN edge-MLP kernel for Trainium2 (8 NeuronCores).

out[e] = W2 @ relu(W1 @ [x[src_e]; x[dst_e]; attr_e] + b1) + b2   for 800K edges.

Strategy (edges sharded 8-way data-parallel, x + weights replicated):
  - x is packed host-side into an SBUF-resident token table (bf16, node i at
    partition i%128, 256B stripe (i//128)*256), loaded once per program run.
  - Endpoint features are fetched with gpsimd.dma_gather(transpose=True)
    reading from SBUF (not HBM): GPSIMD only generates descriptors, the DMA
    engines move the data SBUF->SBUF and write it feature-major
    ([128, n_edges]) -- no random HBM reads in steady state.
  - dma_gather indices are int16 (< 32768), so nodes are split in two
    halves at 25088 (=196*128 ranks) and each core's edge shard is sorted
    host-side into 4 segments by (src>=25088, dst>=25088); within a segment
    the gather source AP (SBUF rank offset 0 or 196) is a compile-time
    constant and indices fit int16.
  - L1 is computed as three accumulated matmuls per 448-edge sub-tile
    (src K=128, dst K=128, attr K=16) into PSUM, relu+bias on the scalar
    engine (-> bf16), L2 as two accumulated matmuls, bias on the vector
    engine (-> bf16), and the transposed output tile is DMA'd to DRAM.
  - Host un-permutes/transposes per-core outputs into the final [E, 128].

The bass program is built after the inputs are seen (kernel compiles per
call); segment sizes are data-dependent but identical across cores (padded to
the max over cores).
"""

import numpy as np
import ml_dtypes

import concourse.bacc as bacc
import concourse.tile as tile
import concourse.mybir as mybir
from concourse import library_config
from concourse.bass_utils import run_bass_kernel_spmd

N_NODES = 50000
N_EDGES = 800000
C = 128      # in_channels
H = 256      # hidden
OUT = 128    # out_channels
HOP = 16
HALF = 25088  # node-id split (multiple of 128) so gather indices fit int16
N_RANKS = 391          # ceil(N_NODES / 128)
N_PAD = N_RANKS * 128  # 50048 token slots in the SBUF x table
N_CORES = 8
NG = 896     # gather tile (edges); 58 SWDGE descs/direction fits the ring
SUB = 448    # matmul sub-tile (edges), NG == 2*SUB
NQ = 4       # SWDGE queues, gathers round-robin

BF16 = mybir.dt.bfloat16
F32 = mybir.dt.float32
I16 = mybir.dt.int16


def _compute_layout(edge_index):
    epc = N_EDGES // N_CORES
    src = edge_index[0].astype(np.int64)
    dst = edge_index[1].astype(np.int64)
    core_perms = []
    core_seg_counts = np.zeros((N_CORES, 4), np.int64)
    for c in range(N_CORES):
        lo, hi = c * epc, (c + 1) * epc
        s, d = src[lo:hi], dst[lo:hi]
        g = (s >= HALF) * 2 + (d >= HALF)
        order = np.argsort(g, kind="stable")
        core_perms.append(lo + order)
        core_seg_counts[c] = np.bincount(g, minlength=4)
    # segments padded to 2*NG so output flushes are whole [128, 4*SUB] groups
    pad_sizes = [int(-(-core_seg_counts[:, g].max() // (2 * NG)) * (2 * NG))
                 for g in range(4)]
    return core_perms, core_seg_counts, pad_sizes


def _build_core_inputs(x, edge_index, edge_attr, W1, b1, W2, b2,
                       core_perms, core_seg_counts, pad_sizes):
    src = edge_index[0].astype(np.int64)
    dst = edge_index[1].astype(np.int64)
    E_pad = sum(pad_sizes)
    # pack x into the SBUF token-table layout: node i -> partition i%128,
    # bf16 columns (i//128)*128 .. +128
    x_pad = np.zeros((N_PAD, C), ml_dtypes.bfloat16)
    x_pad[:N_NODES] = x.astype(ml_dtypes.bfloat16)
    x_bf = np.ascontiguousarray(
        x_pad.reshape(N_RANKS, 128, C).transpose(1, 0, 2).reshape(128, N_RANKS * C))

    w1s = np.stack([np.ascontiguousarray(W1[hc * 128:(hc + 1) * 128, 0:C].T)
                    for hc in range(2)]).astype(ml_dtypes.bfloat16)
    w1d = np.stack([np.ascontiguousarray(W1[hc * 128:(hc + 1) * 128, C:2 * C].T)
                    for hc in range(2)]).astype(ml_dtypes.bfloat16)
    w1a = np.stack([np.ascontiguousarray(W1[hc * 128:(hc + 1) * 128, 2 * C:].T)
                    for hc in range(2)]).astype(ml_dtypes.bfloat16)
    w2 = np.stack([np.ascontiguousarray(W2[:, hc * 128:(hc + 1) * 128].T)
                   for hc in range(2)]).astype(ml_dtypes.bfloat16)
    b1c = np.stack([b1[hc * 128:(hc + 1) * 128].reshape(128, 1)
                    for hc in range(2)]).astype(np.float32)
    b2c = b2.reshape(128, 1).astype(np.float32)

    in_maps, row_maps = [], []
    for c in range(N_CORES):
        perm = core_perms[c]
        cnts = core_seg_counts[c]
        src16 = np.zeros(E_pad, np.int16)
        dst16 = np.zeros(E_pad, np.int16)
        attrT = np.zeros((HOP, E_pad), ml_dtypes.bfloat16)
        rows = np.full(E_pad, -1, np.int64)
        off_in = 0
        off_out = 0
        for g in range(4):
            n = int(cnts[g])
            ids = perm[off_in:off_in + n]
            sl = slice(off_out, off_out + n)
            s_off = HALF if (g >> 1) else 0
            d_off = HALF if (g & 1) else 0
            src16[sl] = (src[ids] - s_off).astype(np.int16)
            dst16[sl] = (dst[ids] - d_off).astype(np.int16)
            attrT[:, sl] = edge_attr[ids].T.astype(ml_dtypes.bfloat16)
            rows[sl] = ids
            off_in += n
            off_out += pad_sizes[g]

        def wrap(a):
            # per-group idx blocks [G, 16, 112]: each NG-call's 896 idxs
            # wrapped into 16 partitions, two calls per group. Small tiles at
            # small offsets keep the Q7 idx-read fast.
            g_count = E_pad // (2 * NG)
            arr = a.reshape(g_count, 2, NG // 16, 16).transpose(0, 3, 1, 2)
            return arr.reshape(g_count, 16, 2 * (NG // 16))

        # combined per-group idx tensor: src blocks then dst blocks, x8
        # partition-replicated -> [G, 128, 224]; one DMA per group feeds all
        # four gathers so the scheduler cannot reorder them (DMASW lane /
        # SWDGE queue phase must stay locked).
        gidx = np.concatenate([wrap(src16), wrap(dst16)], axis=2)
        gidx = np.ascontiguousarray(np.tile(gidx, (1, 8, 1)))

        in_maps.append({
            "x": x_bf,
            "gidx": gidx,
            "attrT": np.ascontiguousarray(attrT),
            "w1s": w1s, "w1d": w1d, "w1a": w1a, "w2": w2,
            "b1": b1c, "b2": b2c,
        })
        row_maps.append(rows)
    return in_maps, row_maps, E_pad


def _build_nc(pad_sizes, reps=1, num_devices=N_CORES, variant="", unroll=False):
    E_pad = sum(pad_sizes)
    nc = bacc.Bacc("TRN2", target_bir_lowering=False, debug=False,
                   num_devices=num_devices, num_swdge_queues=NQ)
    x_d = nc.dram_tensor("x", [128, N_RANKS * C], BF16, kind="ExternalInput")
    n_groups = E_pad // (2 * NG)
    gidx_d = nc.dram_tensor("gidx", [n_groups, 128, 4 * (NG // 16)], I16,
                            kind="ExternalInput")
    attrT = nc.dram_tensor("attrT", [HOP, E_pad], BF16, kind="ExternalInput")
    w1s_d = nc.dram_tensor("w1s", [2, C, 128], BF16, kind="ExternalInput")
    w1d_d = nc.dram_tensor("w1d", [2, C, 128], BF16, kind="ExternalInput")
    w1a_d = nc.dram_tensor("w1a", [2, HOP, 128], BF16, kind="ExternalInput")
    w2_d = nc.dram_tensor("w2", [2, 128, 128], BF16, kind="ExternalInput")
    b1_d = nc.dram_tensor("b1", [2, 128, 1], F32, kind="ExternalInput")
    b2_d = nc.dram_tensor("b2", [128, 1], F32, kind="ExternalInput")
    outT = nc.dram_tensor("outT", [128, E_pad], BF16, kind="ExternalOutput")
    xjunk = None
    if variant == "nogather":
        xjunk = nc.dram_tensor("xjunk", [128, NG], BF16, kind="ExternalInput")

    with tile.TileContext(nc) as tc:
        import contextlib
        with contextlib.ExitStack() as ctx:
            consts = ctx.enter_context(tc.tile_pool(name="consts", bufs=1))
            idxp = ctx.enter_context(tc.tile_pool(name="idxp", bufs=6))
            gp = ctx.enter_context(tc.tile_pool(name="gp", bufs=4))
            ap_ = ctx.enter_context(tc.tile_pool(name="ap", bufs=2))
            hp = ctx.enter_context(tc.tile_pool(name="hp", bufs=10))
            op_ = ctx.enter_context(tc.tile_pool(name="op", bufs=3))
            ps1 = ctx.enter_context(tc.tile_pool(name="ps1", bufs=4, space="PSUM"))
            ps2p = ctx.enter_context(tc.tile_pool(name="ps2", bufs=4, space="PSUM"))

            nc.gpsimd.load_library(library_config.attnmlp)

            w1s_t = [consts.tile([C, 128], BF16, tag=f"w1s{i}", name=f"w1s{i}") for i in range(2)]
            w1d_t = [consts.tile([C, 128], BF16, tag=f"w1d{i}", name=f"w1d{i}") for i in range(2)]
            w1a_t = [consts.tile([HOP, 128], BF16, tag=f"w1a{i}", name=f"w1a{i}") for i in range(2)]
            w2_t = [consts.tile([128, 128], BF16, tag=f"w2{i}", name=f"w2{i}") for i in range(2)]
            b1_t = [consts.tile([128, 1], F32, tag=f"b1{i}", name=f"b1{i}") for i in range(2)]
            b2_t = consts.tile([128, 1], F32, tag="b2")
            for i in range(2):
                nc.sync.dma_start(w1s_t[i][:], w1s_d.ap()[i])
                nc.sync.dma_start(w1d_t[i][:], w1d_d.ap()[i])
                nc.sync.dma_start(w1a_t[i][:], w1a_d.ap()[i])
                nc.sync.dma_start(w2_t[i][:], w2_d.ap()[i])
                nc.sync.dma_start(b1_t[i][:], b1_d.ap()[i])
            nc.sync.dma_start(b2_t[:], b2_d.ap())

            # load the packed x token table into SBUF (split so each DMA
            # descriptor stays under the 64KB SDMA limit)
            x_sb = consts.tile([128, N_RANKS * C], BF16, tag="x_sb")
            n_chunk = 4
            step = N_RANKS * C // n_chunk
            for ci in range(n_chunk):
                sl = slice(ci * step, N_RANKS * C if ci == n_chunk - 1 else (ci + 1) * step)
                nc.sync.dma_start(x_sb[:, sl], x_d.ap()[:, sl])

            x_lo = x_sb[:, 0:N_RANKS * C]
            x_hi = x_sb[:, HALF:N_RANKS * C]

            Relu = mybir.ActivationFunctionType.Relu
            qn = [0]

            def emit_rep():
                seg_start = 0
                for g in range(4):
                    npad = pad_sizes[g]
                    if npad == 0:
                        continue
                    src_tab = x_hi if (g >> 1) else x_lo
                    dst_tab = x_hi if (g & 1) else x_lo
                    e0 = seg_start
                    at = None
                    at_base = 0
                    # process one group of 2*NG = 4*SUB edges at a time;
                    # matmuls are batched per weight (4 subtiles each) so the
                    # PE keeps its stationary weights across 4 streams
                    while e0 < seg_start + npad:
                        gi = e0 // (2 * NG)
                        W = NG // 16
                        gidx_t = idxp.tile([128, 4 * W], I16, tag="gidx")
                        geng = nc.sync if gi % 2 == 0 else nc.scalar
                        geng.dma_start(gidx_t[:], gidx_d.ap()[gi])
                        gt = []
                        for t in range(2):
                            srcg = gp.tile([128, 1, NG], BF16, tag="srcg")
                            dstg = None if variant == "nomm1" else gp.tile(
                                [128, 1, NG], BF16, tag="dstg")
                            if variant == "nogather":
                                nc.sync.dma_start(srcg[:, 0, :], xjunk.ap())
                                nc.sync.dma_start(dstg[:, 0, :], xjunk.ap())
                            else:
                                nc.gpsimd.dma_gather(
                                    srcg[:, :, :], src_tab,
                                    gidx_t[:, t * W:(t + 1) * W],
                                    NG, NG, C, transpose=True,
                                    queue_num=qn[0] % NQ,
                                    sbuf_tokens_per_rank=128,
                                    sbuf_free_dim_per_rank=2 * C)
                                qn[0] += 1
                                if variant != "nomm1":
                                    nc.gpsimd.dma_gather(
                                        dstg[:, :, :], dst_tab,
                                        gidx_t[:, (2 + t) * W:(3 + t) * W],
                                        NG, NG, C, transpose=True,
                                        queue_num=qn[0] % NQ,
                                        sbuf_tokens_per_rank=128,
                                        sbuf_free_dim_per_rank=2 * C)
                                    qn[0] += 1
                            gt.append((srcg, dstg))
                        if at is None or e0 - at_base >= 4 * NG:
                            at = ap_.tile([HOP, 4 * NG], BF16, tag="at")
                            at_base = e0
                            na = min(4 * NG, seg_start + npad - e0)
                            nc.sync.dma_start(at[:, :na], attrT.ap()[:, e0:e0 + na])
                        aoff = e0 - at_base
                        if variant in ("nomm", "nomm1"):
                            e0 += 2 * NG
                            continue

                        # the 4 subtiles of this group as (tile, col-slice)
                        def sub(i):
                            srcg, dstg = gt[i // 2]
                            col = slice((i % 2) * SUB, (i % 2 + 1) * SUB)
                            acol = slice(aoff + (i // 2) * NG + (i % 2) * SUB,
                                         aoff + (i // 2) * NG + (i % 2 + 1) * SUB)
                            return srcg[:, 0, col], dstg[:, 0, col], at[:, acol]

                        h_t = {}
                        for hc in range(2):
                            pss = []
                            for i in range(4):
                                ps = ps1.tile([128, SUB], F32, tag="ps1")
                                nc.tensor.matmul(ps[:], w1s_t[hc][:], sub(i)[0],
                                                 start=True, stop=False)
                                pss.append(ps)
                            for i in range(4):
                                nc.tensor.matmul(pss[i][:], w1d_t[hc][:], sub(i)[1],
                                                 start=False, stop=False)
                            for i in range(4):
                                nc.tensor.matmul(pss[i][:], w1a_t[hc][:], sub(i)[2],
                                                 start=False, stop=True)
                            for i in range(4):
                                ht = hp.tile([128, SUB], BF16, tag="h")
                                nc.scalar.activation(ht[:], pss[i][:], Relu,
                                                     bias=b1_t[hc][:])
                                h_t[hc, i] = ht
                        ps2s = []
                        for i in range(4):
                            ps2 = ps2p.tile([128, SUB], F32, tag="ps2")
                            nc.tensor.matmul(ps2[:], w2_t[0][:], h_t[0, i][:],
                                             start=True, stop=False)
                            ps2s.append(ps2)
                        for i in range(4):
                            nc.tensor.matmul(ps2s[i][:], w2_t[1][:], h_t[1, i][:],
                                             start=False, stop=True)
                        ob = op_.tile([128, 4 * SUB], BF16, tag="ot")
                        for i in range(4):
                            nc.vector.tensor_scalar_add(
                                ob[:, i * SUB:(i + 1) * SUB], ps2s[i][:], b2_t[:])
                        if variant != "noout":
                            eng = nc.sync if (e0 // (2 * NG)) % 2 == 0 else nc.scalar
                            eng.dma_start(
                                outT.ap()[:, e0:e0 + 4 * SUB], ob[:])
                        e0 += 2 * NG
                    seg_start += npad

            if reps == 1:
                emit_rep()
            elif unroll:
                for _ in range(reps):
                    emit_rep()
            else:
                with tc.For_i(0, reps):
                    emit_rep()
    nc.compile()
    return nc


def _assemble_output(results, row_maps):
    out = np.empty((N_EDGES, OUT), np.float32)
    for c in range(N_CORES):
        rows = row_maps[c]
        m = rows >= 0
        out[rows[m]] = results[c]["outT"][:, m].T.astype(np.float32)
    return out


def build_all(x, edge_index, edge_attr, W1, b1, W2, b2, reps=1, variant="",
              unroll=False):
    """Build (nc, in_maps, row_maps) for the given inputs."""
    core_perms, core_seg_counts, pad_sizes = _compute_layout(edge_index)
    in_maps, row_maps, _ = _build_core_inputs(
        x, edge_index, edge_attr, W1, b1, W2, b2,
        core_perms, core_seg_counts, pad_sizes)
    nc = _build_nc(pad_sizes, reps=reps, variant=variant, unroll=unroll)
    if variant == "nogather":
        for im in in_maps:
            im["xjunk"] = np.zeros((128, NG), ml_dtypes.bfloat16)
    return nc, in_maps, row_maps


def kernel(x, edge_index, edge_attr, W1, b1, W2, b2):
    x = np.asarray(x, np.float32)
    edge_index = np.asarray(edge_index)
    edge_attr = np.asarray(edge_attr, np.float32)
    W1 = np.asarray(W1, np.float32)
    b1 = np.asarray(b1, np.float32)
    W2 = np.asarray(W2, np.float32)
    b2 = np.asarray(b2, np.float32)
    assert x.shape == (N_NODES, C) and edge_index.shape == (2, N_EDGES)

    nc, in_maps, row_maps = build_all(x, edge_index, edge_attr, W1, b1, W2, b2)

    last_err = None
    for _attempt in range(3):
        try:
            res = run_bass_kernel_spmd(nc, in_maps, core_ids=list(range(N_CORES)))
            break
        except Exception as e:  # transient device errors: retry
            last_err = e
    else:
        raise last_err
    return _assemble_output(res.results, row_maps)



# revision 7
# speedup vs baseline: 2.0304x; 2.0304x over previous
"""DistanceConv2 GNN edge-MLP kernel for Trainium2 (8 NeuronCores).

out[e] = W2 @ relu(W1 @ [x[src_e]; x[dst_e]; attr_e] + b1) + b2   for 800K edges.

Strategy (edges sharded 8-way data-parallel, x + weights replicated):
  - x is packed host-side into an SBUF-resident token table (bf16, node i at
    partition i%128, 256B stripe (i//128)*256), loaded once per program run.
  - Endpoint features are fetched with gpsimd.dma_gather(transpose=True)
    reading from SBUF (not HBM): GPSIMD only generates descriptors, the DMA
    engines move the data SBUF->SBUF and write it feature-major
    ([128, n_edges]) -- no random HBM reads in steady state.
  - dma_gather indices are int16 (< 32768), so nodes are split in two
    halves at 25088 (=196*128 ranks) and each core's edge shard is sorted
    host-side into 4 segments by (src>=25088, dst>=25088); within a segment
    the gather source AP (SBUF rank offset 0 or 196) is a compile-time
    constant and indices fit int16.
  - L1 is computed as three accumulated matmuls per 448-edge sub-tile
    (src K=128, dst K=128, attr K=16) into PSUM, relu+bias on the scalar
    engine (-> bf16), L2 as two accumulated matmuls, bias on the vector
    engine (-> bf16), and the transposed output tile is DMA'd to DRAM.
  - Host un-permutes/transposes per-core outputs into the final [E, 128].

The bass program is built after the inputs are seen (kernel compiles per
call); segment sizes are data-dependent but identical across cores (padded to
the max over cores).
"""

import numpy as np
import ml_dtypes

import concourse.bacc as bacc
import concourse.tile as tile
import concourse.mybir as mybir
from concourse import library_config
from concourse.bass_utils import run_bass_kernel_spmd

N_NODES = 50000
N_EDGES = 800000
C = 128      # in_channels
H = 256      # hidden
OUT = 128    # out_channels
HOP = 16
HALF = 25088  # node-id split (multiple of 128) so gather indices fit int16
N_RANKS = 391          # ceil(N_NODES / 128)
N_PAD = N_RANKS * 128  # 50048 token slots in the SBUF x table
N_CORES = 8
NG = 896     # gather tile (edges); 58 SWDGE descs/direction fits the ring
SUB = 448    # matmul sub-tile (edges), NG == 2*SUB
NQ = 4       # SWDGE queues, gathers round-robin

BF16 = mybir.dt.bfloat16
F32 = mybir.dt.float32
I16 = mybir.dt.int16


def _compute_layout(edge_index):
    epc = N_EDGES // N_CORES
    src = edge_index[0].astype(np.int64)
    dst = edge_index[1].astype(np.int64)
    core_perms = []
    core_seg_counts = np.zeros((N_CORES, 4), np.int64)
    for c in range(N_CORES):
        lo, hi = c * epc, (c + 1) * epc
        s, d = src[lo:hi], dst[lo:hi]
        g = (s >= HALF) * 2 + (d >= HALF)
        order = np.argsort(g, kind="stable")
        core_perms.append(lo + order)
        core_seg_counts[c] = np.bincount(g, minlength=4)
    # segments padded to 2*NG so output flushes are whole [128, 4*SUB] groups
    pad_sizes = [int(-(-core_seg_counts[:, g].max() // (2 * NG)) * (2 * NG))
                 for g in range(4)]
    return core_perms, core_seg_counts, pad_sizes


def _build_core_inputs(x, edge_index, edge_attr, W1, b1, W2, b2,
                       core_perms, core_seg_counts, pad_sizes):
    src = edge_index[0].astype(np.int64)
    dst = edge_index[1].astype(np.int64)
    E_pad = sum(pad_sizes)
    # pack x into the SBUF token-table layout: node i -> partition i%128,
    # bf16 columns (i//128)*128 .. +128
    x_pad = np.zeros((N_PAD, C), ml_dtypes.bfloat16)
    x_pad[:N_NODES] = x.astype(ml_dtypes.bfloat16)
    x_bf = np.ascontiguousarray(
        x_pad.reshape(N_RANKS, 128, C).transpose(1, 0, 2).reshape(128, N_RANKS * C))

    w1s = np.stack([np.ascontiguousarray(W1[hc * 128:(hc + 1) * 128, 0:C].T)
                    for hc in range(2)]).astype(ml_dtypes.bfloat16)
    w1d = np.stack([np.ascontiguousarray(W1[hc * 128:(hc + 1) * 128, C:2 * C].T)
                    for hc in range(2)]).astype(ml_dtypes.bfloat16)
    w1a = np.stack([np.ascontiguousarray(W1[hc * 128:(hc + 1) * 128, 2 * C:].T)
                    for hc in range(2)]).astype(ml_dtypes.bfloat16)
    w2 = np.stack([np.ascontiguousarray(W2[:, hc * 128:(hc + 1) * 128].T)
                   for hc in range(2)]).astype(ml_dtypes.bfloat16)
    b1c = np.stack([b1[hc * 128:(hc + 1) * 128].reshape(128, 1)
                    for hc in range(2)]).astype(np.float32)
    b2c = b2.reshape(128, 1).astype(np.float32)

    in_maps, row_maps = [], []
    for c in range(N_CORES):
        perm = core_perms[c]
        cnts = core_seg_counts[c]
        src16 = np.zeros(E_pad, np.int16)
        dst16 = np.zeros(E_pad, np.int16)
        attrT = np.zeros((HOP, E_pad), ml_dtypes.bfloat16)
        rows = np.full(E_pad, -1, np.int64)
        off_in = 0
        off_out = 0
        for g in range(4):
            n = int(cnts[g])
            ids = perm[off_in:off_in + n]
            sl = slice(off_out, off_out + n)
            s_off = HALF if (g >> 1) else 0
            d_off = HALF if (g & 1) else 0
            src16[sl] = (src[ids] - s_off).astype(np.int16)
            dst16[sl] = (dst[ids] - d_off).astype(np.int16)
            attrT[:, sl] = edge_attr[ids].T.astype(ml_dtypes.bfloat16)
            rows[sl] = ids
            off_in += n
            off_out += pad_sizes[g]

        def wrap(a):
            # per-group idx blocks [G, 16, 112]: each NG-call's 896 idxs
            # wrapped into 16 partitions, two calls per group. Small tiles at
            # small offsets keep the Q7 idx-read fast.
            g_count = E_pad // (2 * NG)
            arr = a.reshape(g_count, 2, NG // 16, 16).transpose(0, 3, 1, 2)
            return arr.reshape(g_count, 16, 2 * (NG // 16))

        # combined per-group idx tensor: src blocks then dst blocks, x8
        # partition-replicated -> [G, 128, 224]; one DMA per group feeds all
        # four gathers so the scheduler cannot reorder them (DMASW lane /
        # SWDGE queue phase must stay locked).
        gidx = np.concatenate([wrap(src16), wrap(dst16)], axis=2)
        gidx = np.ascontiguousarray(np.tile(gidx, (1, 8, 1)))

        in_maps.append({
            "x": x_bf,
            "gidx": gidx,
            "attrT": np.ascontiguousarray(attrT),
            "w1s": w1s, "w1d": w1d, "w1a": w1a, "w2": w2,
            "b1": b1c, "b2": b2c,
        })
        row_maps.append(rows)
    return in_maps, row_maps, E_pad


def _build_nc(pad_sizes, reps=1, num_devices=N_CORES, variant="", unroll=False):
    E_pad = sum(pad_sizes)
    nc = bacc.Bacc("TRN2", target_bir_lowering=False, debug=False,
                   num_devices=num_devices, num_swdge_queues=NQ)
    x_d = nc.dram_tensor("x", [128, N_RANKS * C], BF16, kind="ExternalInput")
    n_groups = E_pad // (2 * NG)
    gidx_d = nc.dram_tensor("gidx", [n_groups, 128, 4 * (NG // 16)], I16,
                            kind="ExternalInput")
    attrT = nc.dram_tensor("attrT", [HOP, E_pad], BF16, kind="ExternalInput")
    w1s_d = nc.dram_tensor("w1s", [2, C, 128], BF16, kind="ExternalInput")
    w1d_d = nc.dram_tensor("w1d", [2, C, 128], BF16, kind="ExternalInput")
    w1a_d = nc.dram_tensor("w1a", [2, HOP, 128], BF16, kind="ExternalInput")
    w2_d = nc.dram_tensor("w2", [2, 128, 128], BF16, kind="ExternalInput")
    b1_d = nc.dram_tensor("b1", [2, 128, 1], F32, kind="ExternalInput")
    b2_d = nc.dram_tensor("b2", [128, 1], F32, kind="ExternalInput")
    outT = nc.dram_tensor("outT", [128, E_pad], BF16, kind="ExternalOutput")
    xjunk = None
    if variant == "nogather":
        xjunk = nc.dram_tensor("xjunk", [128, NG], BF16, kind="ExternalInput")

    with tile.TileContext(nc) as tc:
        import contextlib
        with contextlib.ExitStack() as ctx:
            consts = ctx.enter_context(tc.tile_pool(name="consts", bufs=1))
            idxp = ctx.enter_context(tc.tile_pool(name="idxp", bufs=6))
            gp = ctx.enter_context(tc.tile_pool(name="gp", bufs=8))
            ap_ = ctx.enter_context(tc.tile_pool(name="ap", bufs=2))
            hp = ctx.enter_context(tc.tile_pool(name="hp", bufs=10))
            op_ = ctx.enter_context(tc.tile_pool(name="op", bufs=3))
            ps1 = ctx.enter_context(tc.tile_pool(name="ps1", bufs=4, space="PSUM"))
            ps2p = ctx.enter_context(tc.tile_pool(name="ps2", bufs=4, space="PSUM"))

            nc.gpsimd.load_library(library_config.attnmlp)

            w1s_t = [consts.tile([C, 128], BF16, tag=f"w1s{i}", name=f"w1s{i}") for i in range(2)]
            w1d_t = [consts.tile([C, 128], BF16, tag=f"w1d{i}", name=f"w1d{i}") for i in range(2)]
            w1a_t = [consts.tile([HOP, 128], BF16, tag=f"w1a{i}", name=f"w1a{i}") for i in range(2)]
            w2_t = [consts.tile([128, 128], BF16, tag=f"w2{i}", name=f"w2{i}") for i in range(2)]
            b1_t = [consts.tile([128, 1], F32, tag=f"b1{i}", name=f"b1{i}") for i in range(2)]
            b2_t = consts.tile([128, 1], F32, tag="b2")
            for i in range(2):
                nc.sync.dma_start(w1s_t[i][:], w1s_d.ap()[i])
                nc.sync.dma_start(w1d_t[i][:], w1d_d.ap()[i])
                nc.sync.dma_start(w1a_t[i][:], w1a_d.ap()[i])
                nc.sync.dma_start(w2_t[i][:], w2_d.ap()[i])
                nc.sync.dma_start(b1_t[i][:], b1_d.ap()[i])
            nc.sync.dma_start(b2_t[:], b2_d.ap())

            # load the packed x token table into SBUF (split so each DMA
            # descriptor stays under the 64KB SDMA limit)
            x_sb = consts.tile([128, N_RANKS * C], BF16, tag="x_sb")
            n_chunk = 4
            step = N_RANKS * C // n_chunk
            for ci in range(n_chunk):
                sl = slice(ci * step, N_RANKS * C if ci == n_chunk - 1 else (ci + 1) * step)
                nc.sync.dma_start(x_sb[:, sl], x_d.ap()[:, sl])

            x_lo = x_sb[:, 0:N_RANKS * C]
            x_hi = x_sb[:, HALF:N_RANKS * C]

            Relu = mybir.ActivationFunctionType.Relu
            qn = [0]

            def emit_rep():
                seg_start = 0
                for g in range(4):
                    npad = pad_sizes[g]
                    if npad == 0:
                        continue
                    src_tab = x_hi if (g >> 1) else x_lo
                    dst_tab = x_hi if (g & 1) else x_lo
                    e0 = seg_start
                    at = None
                    at_base = 0
                    # process one group of 2*NG = 4*SUB edges at a time;
                    # matmuls are batched per weight (4 subtiles each) so the
                    # PE keeps its stationary weights across 4 streams
                    while e0 < seg_start + npad:
                        gi = e0 // (2 * NG)
                        W = NG // 16
                        gidx_t = idxp.tile([128, 4 * W], I16, tag="gidx")
                        geng = nc.sync if gi % 2 == 0 else nc.scalar
                        geng.dma_start(gidx_t[:], gidx_d.ap()[gi])
                        gt = []
                        for t in range(2):
                            srcg = gp.tile([128, 1, NG], BF16, tag="srcg",
                                           name="srcg")
                            dstg = None if variant == "nomm1" else gp.tile(
                                [128, 1, NG], BF16, tag="srcg", name="dstg")
                            if variant == "nogather":
                                nc.sync.dma_start(srcg[:, 0, :], xjunk.ap())
                                nc.sync.dma_start(dstg[:, 0, :], xjunk.ap())
                            else:
                                nc.gpsimd.dma_gather(
                                    srcg[:, :, :], src_tab,
                                    gidx_t[:, t * W:(t + 1) * W],
                                    NG, NG, C, transpose=True,
                                    queue_num=qn[0] % NQ,
                                    sbuf_tokens_per_rank=128,
                                    sbuf_free_dim_per_rank=2 * C)
                                qn[0] += 1
                                if variant != "nomm1":
                                    nc.gpsimd.dma_gather(
                                        dstg[:, :, :], dst_tab,
                                        gidx_t[:, (2 + t) * W:(3 + t) * W],
                                        NG, NG, C, transpose=True,
                                        queue_num=qn[0] % NQ,
                                        sbuf_tokens_per_rank=128,
                                        sbuf_free_dim_per_rank=2 * C)
                                    qn[0] += 1
                            gt.append((srcg, dstg))
                        if at is None or e0 - at_base >= 4 * NG:
                            at = ap_.tile([HOP, 4 * NG], BF16, tag="at")
                            at_base = e0
                            na = min(4 * NG, seg_start + npad - e0)
                            nc.sync.dma_start(at[:, :na], attrT.ap()[:, e0:e0 + na])
                        aoff = e0 - at_base
                        if variant in ("nomm", "nomm1"):
                            e0 += 2 * NG
                            continue

                        # the 4 subtiles of this group as (tile, col-slice)
                        def sub(i):
                            srcg, dstg = gt[i // 2]
                            col = slice((i % 2) * SUB, (i % 2 + 1) * SUB)
                            acol = slice(aoff + (i // 2) * NG + (i % 2) * SUB,
                                         aoff + (i // 2) * NG + (i % 2 + 1) * SUB)
                            return srcg[:, 0, col], dstg[:, 0, col], at[:, acol]

                        h_t = {}
                        for hc in range(2):
                            pss = []
                            for i in range(4):
                                ps = ps1.tile([128, SUB], F32, tag="ps1")
                                nc.tensor.matmul(ps[:], w1s_t[hc][:], sub(i)[0],
                                                 start=True, stop=False)
                                pss.append(ps)
                            for i in range(4):
                                nc.tensor.matmul(pss[i][:], w1d_t[hc][:], sub(i)[1],
                                                 start=False, stop=False)
                            for i in range(4):
                                nc.tensor.matmul(pss[i][:], w1a_t[hc][:], sub(i)[2],
                                                 start=False, stop=True)
                            for i in range(4):
                                ht = hp.tile([128, SUB], BF16, tag="h")
                                nc.scalar.activation(ht[:], pss[i][:], Relu,
                                                     bias=b1_t[hc][:])
                                h_t[hc, i] = ht
                        ps2s = []
                        for i in range(4):
                            ps2 = ps2p.tile([128, SUB], F32, tag="ps2")
                            nc.tensor.matmul(ps2[:], w2_t[0][:], h_t[0, i][:],
                                             start=True, stop=False)
                            ps2s.append(ps2)
                        for i in range(4):
                            nc.tensor.matmul(ps2s[i][:], w2_t[1][:], h_t[1, i][:],
                                             start=False, stop=True)
                        ob = op_.tile([128, 4 * SUB], BF16, tag="ot")
                        for i in range(4):
                            nc.vector.tensor_scalar_add(
                                ob[:, i * SUB:(i + 1) * SUB], ps2s[i][:], b2_t[:])
                        if variant != "noout":
                            eng = nc.sync if (e0 // (2 * NG)) % 2 == 0 else nc.scalar
                            eng.dma_start(
                                outT.ap()[:, e0:e0 + 4 * SUB], ob[:])
                        e0 += 2 * NG
                    seg_start += npad

            if reps == 1:
                emit_rep()
            elif unroll:
                for _ in range(reps):
                    emit_rep()
            else:
                with tc.For_i(0, reps):
                    emit_rep()
    nc.compile()
    return nc


def _assemble_output(results, row_maps):
    out = np.empty((N_EDGES, OUT), np.float32)
    for c in range(N_CORES):
        rows = row_maps[c]
        m = rows >= 0
        out[rows[m]] = results[c]["outT"][:, m].T.astype(np.float32)
    return out


def build_all(x, edge_index, edge_attr, W1, b1, W2, b2, reps=1, variant="",
              unroll=False):
    """Build (nc, in_maps, row_maps) for the given inputs."""
    core_perms, core_seg_counts, pad_sizes = _compute_layout(edge_index)
    in_maps, row_maps, _ = _build_core_inputs(
        x, edge_index, edge_attr, W1, b1, W2, b2,
        core_perms, core_seg_counts, pad_sizes)
    nc = _build_nc(pad_sizes, reps=reps, variant=variant, unroll=unroll)
    if variant == "nogather":
        for im in in_maps:
            im["xjunk"] = np.zeros((128, NG), ml_dtypes.bfloat16)
    return nc, in_maps, row_maps


def kernel(x, edge_index, edge_attr, W1, b1, W2, b2):
    x = np.asarray(x, np.float32)
    edge_index = np.asarray(edge_index)
    edge_attr = np.asarray(edge_attr, np.float32)
    W1 = np.asarray(W1, np.float32)
    b1 = np.asarray(b1, np.float32)
    W2 = np.asarray(W2, np.float32)
    b2 = np.asarray(b2, np.float32)
    assert x.shape == (N_NODES, C) and edge_index.shape == (2, N_EDGES)

    nc, in_maps, row_maps = build_all(x, edge_index, edge_attr, W1, b1, W2, b2)

    last_err = None
    for _attempt in range(3):
        try:
            res = run_bass_kernel_spmd(nc, in_maps, core_ids=list(range(N_CORES)))
            break
        except Exception as e:  # transient device errors: retry
            last_err = e
    else:
        raise last_err
    return _assemble_output(res.results, row_maps)

